# revision 1
# baseline (speedup 1.0000x reference)
"""Cross-correlation layer kernel for Trainium2 (Bass/Tile), SPMD over 8 cores.

Problem: out[b, k, t] = sum_c x1[b, c, t] * x2p[b, c, t + 2D - k]
with x2p = zero-pad(x2, D) along time, D = 10, k in [0, 21).

Full shapes: x1, x2: [16, 512, 8192] fp32 -> out: [16, 21, 8192] fp32.

Sharding: pure data parallel over batch. Each of the 8 cores gets 2 batches
and computes its [2, 21, 8192] slice locally; host concatenates.

Per-core algorithm (slab = 2048 time-columns):
  Inputs are cast fp32->bf16 during the DMA load (SWDGE cast path on gpsimd,
  one ~1MB-read DMA per channel-chunk per slab: the Tile framework recycles
  only 8 SWDGE completion sems, so fewer/bigger loads keep the queue fed).
  For each time block of 128 (t0) the PE accumulates over 4 channel chunks
  in fp32 PSUM:
      G[u, jj] = sum_c x1[c, t0+u] * x2p[c, t0+jj],  u in [0,128), jj in [0,148)
  Two adjacent blocks share one PSUM tile ([128, 296] f32, one 2KB bank) and
  one DVE cast stages both into a wide SBUF tile. The needed outputs are the
  21 band diagonals out[20-d, t0+u] = G[u, u+d]; a per-partition skewed read
  is not expressible on-chip (compute-engine and DMA access patterns apply
  the same free offsets to every partition), so staged G is dumped per
  4-block group to a DRAM scratch, where the diagonal becomes a plain
  strided pattern: with row stride SW2, element (u, blk, d) sits at
  (SW2+1)*u + 148*blk + d. A banded 3-dim gather [[SW2+1,128],[148,HB],[1,21]]
  reads ONLY the 21-wide windows (42B runs) so gather traffic is 84 elems/row
  per dump, and the gathered tile [128, (blk, 21)] is already packed: a PE
  transpose (identity matmul) flips to [(blk, d), u], a DVE copy lands it in
  SBUF, and one DMA writes 512B-contiguous runs into out[b, k, :] (negative
  k-stride realizes k = 20 - d).

  Scheduling: engines execute their instruction streams IN ORDER, so each
  extract stage is emitted where its waits are already satisfied: a slab's
  transposes run at the START of the next slab (PE is load-gated there),
  its osb copies and stores at the next slab's end (after all casts, so
  casts never queue behind the tp->gather sem chain and PSUM-starve the
  matmuls); Sync carries dumps+stores, Scalar only gathers, DVE
  casts+copies, gpsimd only loads. The last slab's loads are split in two
  column-chunks (subtile deps) to shorten the serial tail.
"""

import numpy as np

import concourse.bass as bass
import concourse.mybir as mybir
import concourse.tile as tile
from concourse import bacc
from concourse.masks import make_identity

D = 10
K = 2 * D + 1  # 21 displacements

F32 = mybir.dt.float32
F32R = mybir.dt.float32r
BF16 = mybir.dt.bfloat16


def build_nc(
    B, C, T, slab, group, n_cores=8, ldsplit=2, pair=True, edelay=2, band=False,
    lslast=2, ddiv=None,
):
    """Build the per-core Bass program for inputs [B, C, T] -> out [B, K, T].

    ldsplit: column-chunks per slab load (subtile deps let early blocks start)
    pair:    two blocks per PSUM tile, one staging copy per pair
    edelay:  slabs of delay before a slab's extract stage runs
    band:    compute G in two 64-row bands per block ([128p=(band,r), 84]
             PSUM via partition-offset matmuls) so staging/dump traffic is
             84/148 of the full-G variant; 2x matmul count
    """
    assert C % 128 == 0 and T % slab == 0 and slab % 128 == 0
    nblk_slab = slab // 128
    assert nblk_slab % group == 0
    NCC = C // 128  # channel chunks
    NS = T // slab  # slabs per batch
    GW = 84 if band else 148  # staged width per block (band: 64+2D)
    SW = nblk_slab * GW  # staged G width per slab
    GF = group * K  # gathered free width per group (<=128 for PE transpose)
    assert GF <= 128
    in_dt = BF16
    x2w = slab + 2 * D

    nc = bacc.Bacc("TRN2", target_bir_lowering=False, num_devices=n_cores, num_swdge_queues=2)
    x1 = nc.dram_tensor("x1", [B, C, T], F32, kind="ExternalInput")
    x2 = nc.dram_tensor("x2", [B, C, T], F32, kind="ExternalInput")
    out = nc.dram_tensor("out", [B, K, T], F32, kind="ExternalOutput")
    stg_dt = BF16  # staging/dump/gather dtype
    DDIV = ddiv if ddiv else max(1, nblk_slab // 4)  # dumps per slab
    HB = nblk_slab // DDIV  # blocks per dump
    assert HB % group == 0
    SW2 = HB * GW
    # DRAM scratch: per dump, the G tiles concatenated ([128, HB*148])
    gdr = nc.dram_tensor("gscratch", [B, NS, DDIV, 128, SW2], stg_dt)

    with tile.TileContext(nc) as tc:
        with (
            tc.tile_pool(
                name="x1p", bufs=(5 if slab <= 2048 else 2) * NCC
            ) as x1p,
            tc.tile_pool(
                name="x2p", bufs=(5 if slab <= 2048 else 2) * NCC
            ) as x2p,
            tc.tile_pool(name="gsb", bufs=3) as gsbp,
            tc.tile_pool(name="diag", bufs=13) as diagp,
            tc.tile_pool(name="outp", bufs=8) as outp,
            tc.tile_pool(name="const", bufs=1) as constp,
            tc.tile_pool(name="ps", bufs=4 if pair else 6, space="PSUM") as psp,
            tc.tile_pool(name="pst", bufs=4, space="PSUM") as pstp,
        ):
            # identity for PE transposes; built at the END of the first
            # slab's body (it runs on gpsimd and would otherwise delay the
            # first load issues; it's first consumed one slab later)
            ident = constp.tile([128, 128], stg_dt)

            # The extract pipeline for a slab is split into three stages that
            # are emitted at different points of the NEXT slab's body, so
            # each engine's in-order stream only ever waits on finished
            # work: PE transposes at slab start (PE is load-gated there),
            # DVE osb copies after the first dump-group's casts (so matmuls
            # are not PSUM-starved waiting on casts queued behind them),
            # Sync stores at slab end (after all dumps have been issued).
            def extract_tp(dtiles):
                tps = []
                for q in range(DDIV):
                    for gl in range(HB // group):
                        # dtile cols [gl*GF, (gl+1)*GF) are already the
                        # packed [128, (group, K)] band for this group
                        pk = dtiles[q][:, gl * GF : (gl + 1) * GF]
                        tp = pstp.tile([GF, 128], stg_dt, tag="tps")
                        nc.tensor.transpose(tp[:, :], pk, ident[:, :])
                        tps.append(tp)
                return tps

            def extract_osb(tps):
                osbs = []
                for tp in tps:
                    osb = outp.tile([GF, 128], F32, tag="osb")
                    nc.vector.tensor_copy(osb[:, :], tp[:, :])
                    osbs.append(osb)
                return osbs

            def extract_store(osbs, eb, es):
                for i, osb in enumerate(osbs):
                    q, gl = divmod(i, HB // group)
                    # out[b, 20-d, t0 + blkd*128 + u]
                    blk0 = es * nblk_slab + q * HB + gl * group
                    dst = bass.AP(
                        out,
                        (eb * K + 2 * D) * T + blk0 * 128,
                        [[128, group], [-T, K], [1, 128]],
                    )
                    nc.sync.dma_start(dst, osb[:, :])

            pending = []
            for b in range(B):
                for s in range(NS):
                    ts0 = s * slab  # slab start time
                    last = b == B - 1 and s == NS - 1
                    ls = lslast if last else ldsplit  # finer: shorter tail
                    ldchunk = slab // ls
                    # ---- load input slabs (SWDGE: casts fp32->bf16 inline) --
                    # Each load is issued as `ls` column-chunks into the same
                    # tile (all chunk-i DMAs before chunk-i+1); subtile deps
                    # let block matmuls start as soon as the columns they read
                    # have landed.
                    x1t = [
                        x1p.tile([128, slab], in_dt, name="x1s", tag="x1s")
                        for _ in range(NCC)
                    ]
                    x2t = [
                        x2p.tile([128, x2w], in_dt, name="x2s", tag="x2s")
                        for _ in range(NCC)
                    ]
                    # the slab-before-last's transposes run at slab start:
                    # PE is load-gated here, and with a two-slab delay the
                    # gathers they read finished >20us ago, so the PE stream
                    # never stalls on them mid-kernel
                    done = pending.pop(0) if len(pending) == edelay else None
                    tps = extract_tp(done[0]) if done is not None else None
                    # x2 tile covers x2 time range [ts0 - D, ts0 - D + x2w)
                    lo = ts0 - D
                    lo_c = max(0, lo)
                    hi_c = min(T, lo + x2w)
                    for cc in range(NCC):
                        if lo_c > lo:
                            nc.vector.memset(x2t[cc][:, 0 : lo_c - lo], 0.0)
                        if hi_c < lo + x2w:
                            nc.vector.memset(x2t[cc][:, hi_c - lo :], 0.0)
                    # chunk i of x2 must cover all columns read by blocks of
                    # chunk i of x1: block u0 reads x2t[:, u0 : u0+GW], so
                    # split at i*ldchunk + 2D.
                    for li in range(ls):
                        for cc in range(NCC):
                            c0 = cc * 128
                            a1 = li * ldchunk
                            b1 = a1 + ldchunk
                            nc.gpsimd.dma_start(
                                x1t[cc][:, a1:b1],
                                x1[b, c0 : c0 + 128, ts0 + a1 : ts0 + b1],
                            )
                            a2 = 0 if li == 0 else li * ldchunk + 2 * D
                            b2 = x2w if li == ls - 1 else (li + 1) * ldchunk + 2 * D
                            # clip to the in-bounds region [lo_c-lo, hi_c-lo)
                            a2 = max(a2, lo_c - lo)
                            b2 = min(b2, hi_c - lo)
                            if b2 > a2:
                                nc.gpsimd.dma_start(
                                    x2t[cc][:, a2:b2],
                                    x2[b, c0 : c0 + 128, lo + a2 : lo + b2],
                                )

                    # ---- per 128-block: matmuls -> G psum -> staging tile ----
                    # Engine streams are in-order, so sequence each engine so
                    # it never waits on work that isn't already done: Sync
                    # runs all dumps before any store, Scalar all gathers
                    # before any osb copy; transposes/copies/stores batch at
                    # slab end.
                    gsb = gsbp.tile([128, SW], stg_dt, name="gsb", tag="gsb")
                    bstep = 2 if pair else 1
                    dtiles = []
                    for q in range(DDIV):
                        for blk in range(q * HB, (q + 1) * HB, bstep):
                            gps = psp.tile([128, bstep * GW], F32, tag="gps")
                            for sub in range(bstep):
                                u0 = (blk + sub) * 128
                                if band:
                                    # two 64-row bands stacked on partitions:
                                    # gps[64*Bn + r, sub*84 + j] =
                                    #   sum_c x1[c, u0+64*Bn+r]*x2[c, u0+64*Bn+j]
                                    for bn in range(2):
                                        ub = u0 + 64 * bn
                                        for cc in range(NCC):
                                            nc.tensor.matmul(
                                                gps[
                                                    64 * bn : 64 * (bn + 1),
                                                    sub * GW : (sub + 1) * GW,
                                                ],
                                                x1t[cc][:, ub : ub + 64],
                                                x2t[cc][:, ub : ub + GW],
                                                start=(cc == 0),
                                                stop=(cc == NCC - 1),
                                            )
                                else:
                                    for cc in range(NCC):
                                        nc.tensor.matmul(
                                            gps[:, sub * GW : (sub + 1) * GW],
                                            x1t[cc][:, u0 : u0 + 128],
                                            x2t[cc][:, u0 : u0 + GW],
                                            start=(cc == 0),
                                            stop=(cc == NCC - 1),
                                        )
                            nc.vector.tensor_copy(
                                gsb[:, blk * GW : (blk + bstep) * GW], gps[:, :]
                            )
                        # dump this chunk + banded gather: read only the
                        # 21-wide diagonal windows of each block (42B runs)
                        nc.sync.dma_start(
                            gdr[b, s, q], gsb[:, q * SW2 : (q + 1) * SW2]
                        )
                        dtile = diagp.tile(
                            [128, HB * K], stg_dt, name="dt", tag="diag"
                        )
                        base = ((b * NS + s) * DDIV + q) * 128 * SW2
                        if band:
                            # scratch row p holds band p//64: window of row
                            # r = p%64 starts at col r; one 3-dim gather per
                            # band keeps the pattern linear
                            for bn in range(2):
                                src = bass.AP(
                                    gdr,
                                    base + bn * 64 * SW2,
                                    [[SW2 + 1, 64], [GW, HB], [1, K]],
                                )
                                nc.scalar.dma_start(
                                    dtile[64 * bn : 64 * (bn + 1), :], src
                                )
                        else:
                            src = bass.AP(
                                gdr,
                                base,
                                [[SW2 + 1, 128], [GW, HB], [1, K]],
                            )
                            # dtile[u, K*bb + d] = G_bb[u, u+d]
                            nc.scalar.dma_start(dtile[:, :], src)
                        dtiles.append(dtile)
                    if done is not None:
                        # osb copies run AFTER all of this slab's casts on
                        # DVE: casts must not queue behind the previous
                        # slab's tp->gather sem chain (PSUM-starving matmuls)
                        extract_store(extract_osb(tps), done[1], done[2])
                    if b == 0 and s == 0:
                        make_identity(nc, ident[:, :])
                    pending.append((dtiles, b, s))
            # flush the last two slabs' extracts
            for done in pending:
                extract_store(extract_osb(extract_tp(done[0])), done[1], done[2])

    nc.compile()
    return nc


_NC_CACHE = {}


def _get_nc(B, C, T, slab, group, n_cores, ldsplit, pair, edelay, band, lslast, ddiv):
    key = (B, C, T, slab, group, n_cores, ldsplit, pair, edelay, band, lslast, ddiv)
    if key not in _NC_CACHE:
        _NC_CACHE[key] = build_nc(
            B, C, T, slab, group, n_cores=n_cores, ldsplit=ldsplit, pair=pair,
            edelay=edelay, band=band, lslast=lslast, ddiv=ddiv,
        )
    return _NC_CACHE[key]


def run_sharded(
    x1, x2, slab=2048, group=4, ldsplit=1, pair=True, edelay=1, band=False,
    lslast=2, ddiv=None, trace=False, **spmd_kwargs,
):
    """Run the SPMD kernel on 8 cores over full inputs; returns (out, results)."""
    from concourse.bass_utils import run_bass_kernel_spmd

    n_cores = 8
    Bf, C, T = x1.shape
    assert Bf % n_cores == 0
    Bs = Bf // n_cores
    nc = _get_nc(Bs, C, T, slab, group, n_cores, ldsplit, pair, edelay, bool(band), lslast, ddiv)
    in_maps = [
        {
            "x1": np.ascontiguousarray(x1[i * Bs : (i + 1) * Bs]),
            "x2": np.ascontiguousarray(x2[i * Bs : (i + 1) * Bs]),
        }
        for i in range(n_cores)
    ]
    res = run_bass_kernel_spmd(
        nc, in_maps, core_ids=list(range(n_cores)), trace=trace, **spmd_kwargs
    )
    out = np.concatenate([r["out"] for r in res.results], axis=0)
    return out, res


def kernel(x1, x2):
    x1 = np.asarray(x1, dtype=np.float32)
    x2 = np.asarray(x2, dtype=np.float32)
    out, _ = run_sharded(x1, x2)
    return out



# revision 28
# speedup vs baseline: 1.0672x; 1.0672x over previous
"""Cross-correlation layer kernel for Trainium2 (Bass/Tile), SPMD over 8 cores.

Problem: out[b, k, t] = sum_c x1[b, c, t] * x2p[b, c, t + 2D - k]
with x2p = zero-pad(x2, D) along time, D = 10, k in [0, 21).

Full shapes: x1, x2: [16, 512, 8192] fp32 -> out: [16, 21, 8192] fp32.

Sharding: pure data parallel over batch. Each of the 8 cores gets 2 batches
and computes its [2, 21, 8192] slice locally; host concatenates.

Per-core algorithm (slab = 2048 time-columns):
  Inputs are cast fp32->bf16 during the DMA load (SWDGE cast path on gpsimd,
  one ~1MB-read DMA per channel-chunk per slab: the Tile framework recycles
  only 8 SWDGE completion sems, so fewer/bigger loads keep the queue fed).
  For each time block of 128 (t0) the PE accumulates over 4 channel chunks
  in fp32 PSUM:
      G[u, jj] = sum_c x1[c, t0+u] * x2p[c, t0+jj],  u in [0,128), jj in [0,148)
  Two adjacent blocks share one PSUM tile ([128, 296] f32, one 2KB bank) and
  one DVE cast stages both into a wide SBUF tile. The needed outputs are the
  21 band diagonals out[20-d, t0+u] = G[u, u+d]; a per-partition skewed read
  is not expressible on-chip (compute-engine and DMA access patterns apply
  the same free offsets to every partition), so staged G is dumped per
  4-block group to a DRAM scratch, where the diagonal becomes a plain
  strided pattern: with row stride SW2, element (u, blk, d) sits at
  (SW2+1)*u + 148*blk + d. A banded 3-dim gather [[SW2+1,128],[148,HB],[1,21]]
  reads ONLY the 21-wide windows (42B runs) so gather traffic is 84 elems/row
  per dump, and the gathered tile [128, (blk, 21)] is already packed: a PE
  transpose (identity matmul) flips to [(blk, d), u], a DVE copy lands it in
  SBUF, and one DMA writes 512B-contiguous runs into out[b, k, :] (negative
  k-stride realizes k = 20 - d).

  Scheduling: engines execute their instruction streams IN ORDER, so each
  extract stage is emitted where its waits are already satisfied: a slab's
  transposes run at the START of the next slab (PE is load-gated there),
  its osb copies and stores at the next slab's end (after all casts, so
  casts never queue behind the tp->gather sem chain and PSUM-starve the
  matmuls); Sync carries dumps+stores, Scalar only gathers, DVE
  casts+copies, gpsimd only loads. The last slab's loads are split in two
  column-chunks (subtile deps) to shorten the serial tail.
"""

import numpy as np

import concourse.bass as bass
import concourse.mybir as mybir
import concourse.tile as tile
from concourse import bacc
from concourse.masks import make_identity

D = 10
K = 2 * D + 1  # 21 displacements

F32 = mybir.dt.float32
F32R = mybir.dt.float32r
BF16 = mybir.dt.bfloat16


def build_nc(
    B, C, T, slab, group, n_cores=8, ldsplit=2, pair=True, edelay=2, band=False,
    lslast=2, ddiv=None, wide=True, ldbufs=4,
):
    """Build the per-core Bass program for inputs [B, C, T] -> out [B, K, T].

    ldsplit: column-chunks per slab load (subtile deps let early blocks start)
    pair:    two blocks per PSUM tile, one staging copy per pair
    edelay:  slabs of delay before a slab's extract stage runs
    band:    compute G in two 64-row bands per block ([128p=(band,r), 84]
             PSUM via partition-offset matmuls) so staging/dump traffic is
             84/148 of the full-G variant; 2x matmul count
    """
    assert C % 128 == 0 and T % slab == 0 and slab % 128 == 0
    assert not (wide and band)
    nblk_slab = slab // 128
    assert nblk_slab % group == 0
    NCC = C // 128  # channel chunks
    NS = T // slab  # slabs per batch
    GW = 84 if band else 148  # staged width per block (band: 64+2D)
    SW = nblk_slab * GW  # staged G width per slab
    GF = group * K  # gathered free width per group (<=128 for PE transpose)
    assert GF <= 128
    in_dt = BF16
    x2w = slab + 2 * D

    nc = bacc.Bacc("TRN2", target_bir_lowering=False, num_devices=n_cores, num_swdge_queues=2)
    x1 = nc.dram_tensor("x1", [B, C, T], F32, kind="ExternalInput")
    x2 = nc.dram_tensor("x2", [B, C, T], F32, kind="ExternalInput")
    out = nc.dram_tensor("out", [B, K, T], F32, kind="ExternalOutput")
    stg_dt = BF16  # staging/dump/gather dtype
    DDIV = ddiv if ddiv else max(1, nblk_slab // 4)  # dumps per slab
    HB = nblk_slab // DDIV  # blocks per dump
    assert HB % group == 0
    SW2 = HB * GW
    GF2 = HB * K  # gathered elems per row per dump (wide/interleaved mode)
    assert group == HB and GF2 <= 128
    # DRAM scratch: per dump, the G tiles concatenated ([128, HB*148])
    gdr = nc.dram_tensor("gscratch", [B, NS, DDIV, 128, SW2], stg_dt)

    with tile.TileContext(nc) as tc:
        with (
            tc.tile_pool(
                name="x1p", bufs=(ldbufs if slab <= 2048 else 2) * NCC
            ) as x1p,
            tc.tile_pool(
                name="x2p", bufs=(ldbufs if slab <= 2048 else 2) * NCC
            ) as x2p,
            tc.tile_pool(name="gsb", bufs=3) as gsbp,
            tc.tile_pool(name="diag", bufs=13) as diagp,
            tc.tile_pool(name="outp", bufs=8) as outp,
            tc.tile_pool(name="const", bufs=1) as constp,
            tc.tile_pool(name="ps", bufs=4 if pair else 6, space="PSUM") as psp,
            tc.tile_pool(name="pst", bufs=4, space="PSUM") as pstp,
        ):
            # identity for PE transposes; built at the END of the first
            # slab's body (it runs on gpsimd and would otherwise delay the
            # first load issues; it's first consumed one slab later)
            ident = constp.tile([128, 128], stg_dt)

            # The extract pipeline for a slab is split into three stages that
            # are emitted at different points of the NEXT slab's body, so
            # each engine's in-order stream only ever waits on finished
            # work: PE transposes at slab start (PE is load-gated there),
            # DVE osb copies after the first dump-group's casts (so matmuls
            # are not PSUM-starved waiting on casts queued behind them),
            # Sync stores at slab end (after all dumps have been issued).
            def extract_tp(dtiles):
                tps = []
                for q in range(DDIV):
                    for gl in range(HB // group):
                        # dtile cols [gl*GF, (gl+1)*GF) are the packed band
                        # for this group: (k, bb) in wide mode, (g, d) else
                        pk = dtiles[q][:, gl * GF : (gl + 1) * GF]
                        tp = pstp.tile([GF, 128], stg_dt, tag="tps")
                        nc.tensor.transpose(tp[:, :], pk, ident[:, :])
                        tps.append(tp)
                return tps

            def extract_osb(tps):
                osbs = []
                for tp in tps:
                    osb = outp.tile([GF, 128], F32, tag="osb")
                    nc.vector.tensor_copy(osb[:, :], tp[:, :])
                    osbs.append(osb)
                return osbs

            def extract_store(osbs, eb, es):
                for i, osb in enumerate(osbs):
                    q, gl = divmod(i, HB // group)
                    blk0 = es * nblk_slab + q * HB + gl * group
                    if wide:
                        # osb row k*HB + bb -> out[b, k, t0 + (blk0+bb)*128+u]
                        dst = bass.AP(
                            out,
                            eb * K * T + blk0 * 128,
                            [[T, K], [128, HB], [1, 128]],
                        )
                    else:
                        # out[b, 20-d, t0 + blkd*128 + u]
                        dst = bass.AP(
                            out,
                            (eb * K + 2 * D) * T + blk0 * 128,
                            [[128, group], [-T, K], [1, 128]],
                        )
                    nc.sync.dma_start(dst, osb[:, :])

            pending = []
            for b in range(B):
                for s in range(NS):
                    ts0 = s * slab  # slab start time
                    last = b == B - 1 and s == NS - 1
                    ls = lslast if last else ldsplit  # finer: shorter tail
                    ldchunk = slab // ls
                    # ---- load input slabs (SWDGE: casts fp32->bf16 inline) --
                    # Each load is issued as `ls` column-chunks into the same
                    # tile (all chunk-i DMAs before chunk-i+1); subtile deps
                    # let block matmuls start as soon as the columns they read
                    # have landed.
                    x1t = [
                        x1p.tile([128, slab], in_dt, name="x1s", tag="x1s")
                        for _ in range(NCC)
                    ]
                    x2t = [
                        x2p.tile([128, x2w], in_dt, name="x2s", tag="x2s")
                        for _ in range(NCC)
                    ]
                    # the slab-before-last's transposes run at slab start:
                    # PE is load-gated here, and with a two-slab delay the
                    # gathers they read finished >20us ago, so the PE stream
                    # never stalls on them mid-kernel
                    done = pending.pop(0) if len(pending) == edelay else None
                    tps = extract_tp(done[0]) if done is not None else None
                    # x2 tile covers x2 time range [ts0 - D, ts0 - D + x2w)
                    lo = ts0 - D
                    lo_c = max(0, lo)
                    hi_c = min(T, lo + x2w)
                    for cc in range(NCC):
                        if lo_c > lo:
                            nc.vector.memset(x2t[cc][:, 0 : lo_c - lo], 0.0)
                        if hi_c < lo + x2w:
                            nc.vector.memset(x2t[cc][:, hi_c - lo :], 0.0)
                    # chunk i of x2 must cover all columns read by blocks of
                    # chunk i of x1: block u0 reads x2t[:, u0 : u0+GW], so
                    # split at i*ldchunk + 2D.
                    for li in range(ls):
                        for cc in range(NCC):
                            c0 = cc * 128
                            a1 = li * ldchunk
                            b1 = a1 + ldchunk
                            nc.gpsimd.dma_start(
                                x1t[cc][:, a1:b1],
                                x1[b, c0 : c0 + 128, ts0 + a1 : ts0 + b1],
                            )
                            a2 = 0 if li == 0 else li * ldchunk + 2 * D
                            b2 = x2w if li == ls - 1 else (li + 1) * ldchunk + 2 * D
                            # clip to the in-bounds region [lo_c-lo, hi_c-lo)
                            a2 = max(a2, lo_c - lo)
                            b2 = min(b2, hi_c - lo)
                            if b2 > a2:
                                nc.gpsimd.dma_start(
                                    x2t[cc][:, a2:b2],
                                    x2[b, c0 : c0 + 128, lo + a2 : lo + b2],
                                )

                    # ---- per 128-block: matmuls -> G psum -> staging tile ----
                    # Engine streams are in-order, so sequence each engine so
                    # it never waits on work that isn't already done: Sync
                    # runs all dumps before any store, Scalar all gathers
                    # before any osb copy; transposes/copies/stores batch at
                    # slab end.
                    gsb = gsbp.tile([128, SW], stg_dt, name="gsb", tag="gsb")
                    bstep = 2 if pair else 1
                    dtiles = []
                    for q in range(DDIV):
                        for blk in range(q * HB, (q + 1) * HB, bstep):
                            gps = psp.tile([128, bstep * GW], F32, tag="gps")
                            for sub in range(bstep):
                                u0 = (blk + sub) * 128
                                if band:
                                    # two 64-row bands stacked on partitions:
                                    # gps[64*Bn + r, sub*84 + j] =
                                    #   sum_c x1[c, u0+64*Bn+r]*x2[c, u0+64*Bn+j]
                                    for bn in range(2):
                                        ub = u0 + 64 * bn
                                        for cc in range(NCC):
                                            nc.tensor.matmul(
                                                gps[
                                                    64 * bn : 64 * (bn + 1),
                                                    sub * GW : (sub + 1) * GW,
                                                ],
                                                x1t[cc][:, ub : ub + 64],
                                                x2t[cc][:, ub : ub + GW],
                                                start=(cc == 0),
                                                stop=(cc == NCC - 1),
                                            )
                                else:
                                    for cc in range(NCC):
                                        nc.tensor.matmul(
                                            gps[:, sub * GW : (sub + 1) * GW],
                                            x1t[cc][:, u0 : u0 + 128],
                                            x2t[cc][:, u0 : u0 + GW],
                                            start=(cc == 0),
                                            stop=(cc == NCC - 1),
                                        )
                            if wide:
                                # column-interleaved + j-reversed staging:
                                # within group q, scratch col w =
                                # (GW-1-j)*HB + bb, so every row's diag windows
                                # become ONE contiguous (k, bb)-packed GF2-elem
                                # run (k = 20-d ascending) at skew SW2-HB
                                gv = gsb[
                                    :, q * SW2 : (q + 1) * SW2
                                ].rearrange("p (j h) -> p j h", h=HB)
                                for sub in range(bstep):
                                    nc.vector.tensor_copy(
                                        gv[:, ::-1, blk - q * HB + sub],
                                        gps[:, sub * GW : (sub + 1) * GW],
                                    )
                            else:
                                nc.vector.tensor_copy(
                                    gsb[:, blk * GW : (blk + bstep) * GW],
                                    gps[:, :],
                                )
                        # dump this chunk + banded gather: read only the
                        # 21-wide diagonal windows of each block (42B runs)
                        nc.sync.dma_start(
                            gdr[b, s, q], gsb[:, q * SW2 : (q + 1) * SW2]
                        )
                        dtile = diagp.tile(
                            [128, HB * K], stg_dt, name="dt", tag="diag"
                        )
                        base = ((b * NS + s) * DDIV + q) * 128 * SW2
                        if wide:
                            # interleaved scratch: row u's whole band is the
                            # GF2-elem run at (SW2-HB)*u + 127*HB — one 168B
                            # descriptor per row instead of HB 42B ones
                            # (tiny-run gathers starve behind load packets on
                            # the shared SDMA engines: ~40us/slab mid-load)
                            src = bass.AP(
                                gdr,
                                base + 127 * HB,
                                [[SW2 - HB, 128], [1, GF2]],
                            )
                            # dtile[u, HB*k + bb] = G_bb[u, u + 20 - k]
                            nc.scalar.dma_start(dtile[:, :], src)
                        elif band:
                            # scratch row p holds band p//64: window of row
                            # r = p%64 starts at col r; one 3-dim gather per
                            # band keeps the pattern linear
                            for bn in range(2):
                                src = bass.AP(
                                    gdr,
                                    base + bn * 64 * SW2,
                                    [[SW2 + 1, 64], [GW, HB], [1, K]],
                                )
                                nc.scalar.dma_start(
                                    dtile[64 * bn : 64 * (bn + 1), :], src
                                )
                        else:
                            src = bass.AP(
                                gdr,
                                base,
                                [[SW2 + 1, 128], [GW, HB], [1, K]],
                            )
                            # dtile[u, K*bb + d] = G_bb[u, u+d]
                            nc.scalar.dma_start(dtile[:, :], src)
                        dtiles.append(dtile)
                    if done is not None:
                        # osb copies run AFTER all of this slab's casts on
                        # DVE: casts must not queue behind the previous
                        # slab's tp->gather sem chain (PSUM-starving matmuls)
                        extract_store(extract_osb(tps), done[1], done[2])
                    if b == 0 and s == 0:
                        make_identity(nc, ident[:, :])
                    pending.append((dtiles, b, s))
            # flush the last two slabs' extracts
            for done in pending:
                extract_store(extract_osb(extract_tp(done[0])), done[1], done[2])

    nc.compile()
    return nc


_NC_CACHE = {}


def _get_nc(B, C, T, slab, group, n_cores, ldsplit, pair, edelay, band, lslast,
            ddiv, wide, ldbufs):
    key = (B, C, T, slab, group, n_cores, ldsplit, pair, edelay, band, lslast,
           ddiv, wide, ldbufs)
    if key not in _NC_CACHE:
        _NC_CACHE[key] = build_nc(
            B, C, T, slab, group, n_cores=n_cores, ldsplit=ldsplit, pair=pair,
            edelay=edelay, band=band, lslast=lslast, ddiv=ddiv, wide=wide,
            ldbufs=ldbufs,
        )
    return _NC_CACHE[key]


def run_sharded(
    x1, x2, slab=2048, group=4, ldsplit=1, pair=True, edelay=1, band=False,
    lslast=2, ddiv=None, wide=True, ldbufs=4, trace=False, **spmd_kwargs,
):
    """Run the SPMD kernel on 8 cores over full inputs; returns (out, results)."""
    from concourse.bass_utils import run_bass_kernel_spmd

    n_cores = 8
    Bf, C, T = x1.shape
    assert Bf % n_cores == 0
    Bs = Bf // n_cores
    nc = _get_nc(Bs, C, T, slab, group, n_cores, ldsplit, pair, edelay,
                 bool(band), lslast, ddiv, bool(wide), ldbufs)
    in_maps = [
        {
            "x1": np.ascontiguousarray(x1[i * Bs : (i + 1) * Bs]),
            "x2": np.ascontiguousarray(x2[i * Bs : (i + 1) * Bs]),
        }
        for i in range(n_cores)
    ]
    res = run_bass_kernel_spmd(
        nc, in_maps, core_ids=list(range(n_cores)), trace=trace, **spmd_kwargs
    )
    out = np.concatenate([r["out"] for r in res.results], axis=0)
    return out, res


def kernel(x1, x2):
    x1 = np.asarray(x1, dtype=np.float32)
    x2 = np.asarray(x2, dtype=np.float32)
    out, _ = run_sharded(x1, x2)
    return out

